# revision 2
# baseline (speedup 1.0000x reference)
"""Trainium2 Bass kernel for BinOverlapPredictionFromMaxProj (segment max + masked mean).

Full computation:
  ptm: (32, 8, 30, 1, 72, 72) f32, mem_mask: (32, 8, 30) bool
  n = 32*8 = 256 rows; per row: max over 5184-feature axis per mem (30), then
  masked mean over mems -> out (256,) f32.

Sharding: data-parallel over the 256 fused rows across 8 cores (32 rows each).
Per core: 960 segments x 5184 features (~19.9 MB) -> memory-bound.

V2 design notes (on top of the V1 pair-aligned layout):

The 19.9 MB stream rides one SWDGE queue that fans descriptors over the
core's 16 DMA engines. Engine 15 (E79 on nc4) also serves the notification /
HW-dynamic queues and runs ~22% slower in bursts; with uniform round-robin it
builds an ~11us backlog that gates every load's completion semaphore (the
last data landed at ~68us instead of ~57us).

SWDGE lane-assignment rule (measured): for an n-descriptor DMA,
descs_per_lane = smallest divisor of n >= ceil(n/16); nlanes = n/d; lanes
are taken contiguously from a per-queue cursor that advances by nlanes mod
16 and persists across instructions (sem packets don't advance it).

Exploits:
  - 120-desc loads use 15 lanes, skipping (cursor+15): with the cursor parked
    at lane 0, every pair's main load skips the slow engine 15.
  - 4-byte-descriptor re-reads of `consts` serve as free cursor repairs
    (9 descs, +9) and as 16-desc all-lane "cover" loads: per-lane FIFO means
    a cover's completion implies every earlier descriptor on every lane has
    drained, so each pair's reduce can carry ONE sem wait (the cover's)
    instead of one per writer (walrus allows a single attached sync wait).
    A post-pass rewrites the tile-assigned multi-waits accordingly.
  - Pair structure (even: A120/L8/R9/V16, odd: A120/R9/L8/V16) advances the
    cursor by exactly 32 = 0 mod 16 per pair, and gives engine 15 one
    20.7KB descriptor per pair instead of eight.

Tail: col 13 is loaded as two half-columns (112+16-desc second half) so the
final delivery-gated reduce is ~1.4us instead of ~2.8us, and the
pairmax/mask/rowsum for segments 0-5 runs mid-stream; only segment 6 and the
final matmul+mean remain after the last byte. 1/count is computed on host
and shipped in m1 (drops the DVE reciprocal and its table load).
"""

import sys

import numpy as np

if "/opt/trn_rl_repo" not in sys.path:
    sys.path.insert(0, "/opt/trn_rl_repo")

NCORES = 8
NF, NS, NMEM, FEAT = 32, 8, 30, 5184
N = NF * NS  # 256
ROWS = N // NCORES  # 32 rows per core
SEGS = ROWS * NMEM  # 960 segments per core
PPART = 128  # partitions
HALF = FEAT // 2  # 2592 floats per half-segment
HPP = SEGS * 2 // PPART  # 15 half-segments per partition
NWHOLE = 7  # whole segments per partition (cols 0..13)
NPAIR = 6  # column-pairs loaded with the skew structure (cols 0..11)
NC_ = PPART + ROWS + NWHOLE  # consts free dim: ident | w1 | maskA

_NC_CACHE = {}


def _nlanes(n):
    """Lanes used by an n-descriptor SWDGE DMA (measured ucode rule)."""
    need = -(-n // 16)
    for d in range(need, n + 1):
        if n % d == 0:
            return n // d
    return 1


def _patch_tile_drain():
    """Split the kernel-tail Drain's semaphore waits into standalone wait_ge
    instructions (one wait per instruction), to fit the walrus per-instruction
    sync-wait limit."""
    import concourse.tile as tile
    from concourse.vector_clock import ScopedClock

    if getattr(tile.TileContext._drain_and_barrier, "_single_wait_patch", False):
        return

    def _drain_and_barrier(self, tick_clock, wait_clock):
        drain_inst = self.nc.sync.drain()
        wait_clock.add_sem_waits(
            drain_inst.ins, ScopedClock({None: tick_clock.global_clock})
        )
        si = drain_inst.ins.sync_info
        waits = list(si.on_wait) if si is not None else []
        if len(waits) > 1:
            si.on_wait = [waits[0]]
            by_name = {h.name: h for h in self.sems.allocated().values()}
            for w in waits[1:]:
                self.nc.sync.wait_ge(by_name[w.ant_name], w.wait_value)

        self.nc.all_engine_barrier()
        assert self.sems is not None
        popped = self.nc._tile_sem_poison_stack.pop()
        assert popped is self._sem_poison
        self.nc.clear_and_free_semaphores(list(self.sems.allocated().values()))

    _drain_and_barrier._single_wait_patch = True
    tile.TileContext._drain_and_barrier = _drain_and_barrier


def _rewrite_cover_waits(nc, cover_map, merge_rest=True):
    """Post-pass: point each recorded reduce's sem waits at its cover load.

    cover_map: list of (cover_dma_bass_inst, [reduce_bass_inst, ...]).
    Per-lane FIFO on the SWDGE queue makes `cover delivered` imply `all
    earlier descriptors on every lane delivered`, so a single wait on the
    cover's accumulated sem value is a sound replacement for the
    tile-assigned one-wait-per-writer set (walrus allows only one).
    """
    # Accumulate DMA completion-sem values in program order.
    acc = {}
    cover_val = {}
    cover_ids = {id(c.ins): c for c, _ in cover_map}
    for fn in nc.m.functions:
        for b in fn.blocks:
            for ins in b.instructions:
                if type(ins).__name__ != "InstDMACopy":
                    continue
                si = ins.sync_info
                if si is None or not si.on_update:
                    continue
                for u in si.on_update:
                    if not u.ant_name.startswith("DMASW"):
                        continue
                    acc[u.id] = acc.get(u.id, 0) + u.update_value
                    if id(ins) in cover_ids:
                        cover_val[id(ins)] = (u.ant_name, u.id, acc[u.id])
    for cover, reds in cover_map:
        key = id(cover.ins)
        assert key in cover_val, f"cover {cover.ins.name} got no DMASW sem"
        ant_name, sem_id, val = cover_val[key]
        for r in reds:
            si = r.ins.sync_info
            assert si is not None and si.on_wait, f"{r.ins.name} has no waits"
            w = si.on_wait[0]
            w.ant_name = ant_name
            w.id = sem_id
            w.wait_value = val
            si.on_wait = [w]
    if merge_rest:
        for fn in nc.m.functions:
            for b in fn.blocks:
                for ins in b.instructions:
                    si = ins.sync_info
                    if si is None or len(si.on_wait) <= 1:
                        continue
                    waits = list(si.on_wait)
                    names = {w.ant_name for w in waits}
                    assert len(names) == 1, (
                        f"unresolved multi-sem wait on {ins.name}: "
                        f"{[(w.ant_name, w.wait_value) for w in waits]}"
                    )
                    best = max(waits, key=lambda w: w.wait_value)
                    si.on_wait = [best]


def _build_nc():
    import concourse.bass as bass
    import concourse.tile as tile
    from concourse import mybir
    from concourse.bass import MemorySpace

    _patch_tile_drain()

    f32 = mybir.dt.float32
    X = mybir.AxisListType.X

    nc = bass.Bass("TRN2")
    ptm = nc.dram_tensor("ptm", [PPART, HPP, HALF], f32, kind="ExternalInput")
    consts = nc.dram_tensor("consts", [PPART, NC_], f32, kind="ExternalInput")
    m1 = nc.dram_tensor("m1", [1, 2 * ROWS + ROWS], f32, kind="ExternalInput")
    out = nc.dram_tensor("out", [1, ROWS], f32, kind="ExternalOutput")

    cover_map = []  # (cover dma inst, [reduces to rewrite])
    cursor = 0  # SWDGE lane cursor (relative; engine 15 = cursor 15)

    def q0(dst, src, ndesc, expect_adv):
        nonlocal cursor
        inst = nc.gpsimd.dma_start(out=dst, in_=src)
        adv = _nlanes(ndesc) % 16
        assert adv == expect_adv % 16, (ndesc, adv, expect_adv)
        cursor = (cursor + adv) % 16
        return inst

    with tile.TileContext(nc) as tc:
        with (
            tc.tile_pool(name="data", bufs=1) as dpool,
            tc.tile_pool(name="small", bufs=1) as spool,
            tc.tile_pool(name="psum", bufs=1, space=MemorySpace.PSUM) as ppool,
        ):
            # --- early loads -------------------------------------------------
            const_t = spool.tile([PPART, NC_], f32)
            q0(const_t[:], consts[:], 128, 0)
            ident_v = const_t[:, 0:PPART]
            w1_v = const_t[:, PPART : PPART + ROWS]
            maskA_v = const_t[:, PPART + ROWS : NC_]

            m1_t = spool.tile([1, 2 * ROWS + ROWS], f32)
            nc.scalar.dma_start(out=m1_t[:], in_=m1[:])
            maskS2_v = m1_t[0:1, 0 : 2 * ROWS]
            rcntT_v = m1_t[0:1, 2 * ROWS : 2 * ROWS + ROWS]

            # junk tiles for repair/cover reads (4B descriptors)
            jr = spool.tile([16, 16], f32)
            jread = spool.tile([1, 8], f32)

            # PE warmup: touch const tile so later PE ops carry one data wait.
            warm = ppool.tile([1, ROWS], f32)
            nc.tensor.matmul(warm[:], const_t[:, 0:1], const_t[:, 0:ROWS],
                             start=True, stop=True)

            # Stray (half-segment) column, full uniform load.
            dS = dpool.tile([PPART, 1, HALF], f32, name="dataS", tag="dataS")
            q0(dS[:], ptm[:, HPP - 1 : HPP, :], 128, 0)
            statS = spool.tile([PPART, 1], f32)
            nc.vector.reduce_max(out=statS[:], in_=dS[:], axis=X)

            # DVE warm-touch + power-ramp burn in the pre-stream window.
            touch = spool.tile([1, 1], f32)
            nc.vector.tensor_copy(out=touch[:], in_=const_t[0:1, 0:1])
            touch2 = spool.tile([1, 1], f32)
            nc.vector.tensor_copy(out=touch2[:], in_=m1_t[0:1, 0:1])
            burn = spool.tile([PPART, 1], f32)
            nc.vector.reduce_max(out=burn[:], in_=const_t[:], axis=X)
            nc.vector.reduce_max(out=burn[:], in_=const_t[:], axis=X)

            # PE transpose of the stray half-maxes to one partition.
            strayP = ppool.tile([1, PPART], f32)
            nc.tensor.transpose(strayP[:], statS[:], ident_v)

            # --- main skewed pair stream ------------------------------------
            stats12 = spool.tile([PPART, 2 * NPAIR], f32)
            straysum = spool.tile([1, ROWS], f32)
            pair_tiles = []
            for k in range(NPAIR):
                d = dpool.tile([PPART, 2, HALF], f32, name=f"dP{k}", tag=f"dP{k}")
                pair_tiles.append(d)
                src = ptm[:, 2 * k : 2 * k + 2, :]
                assert cursor == 0, cursor
                q0(d[0:120], src[0:120], 120, 15)  # skip lane 15
                if k % 2 == 0:
                    q0(d[120:128], src[120:128], 8, 8)   # lane15 + 0-6
                    q0(jr[0:9, k : k + 1], consts[0:9, 0:1], 9, 9)
                else:
                    q0(jr[0:9, k : k + 1], consts[0:9, 0:1], 9, 9)
                    q0(d[120:128], src[120:128], 8, 8)   # lanes 8-15
                cov = q0(jr[0:16, 8 + k : 9 + k], consts[0:16, 0:1], 16, 0)
                # Reader forces a completion sem onto the cover load.
                cread = nc.vector.tensor_copy(
                    out=jread[0:1, k : k + 1], in_=jr[0:1, 8 + k : 9 + k]
                )
                red = nc.vector.reduce_max(
                    out=stats12[:, 2 * k : 2 * k + 2], in_=d[:], axis=X
                )
                cover_map.append((cov, [cread, red]))

                if k == 1:
                    # Stray path on DVE, mid-stream (off critical path).
                    strayC = spool.tile([1, PPART], f32)
                    nc.vector.tensor_copy(out=strayC[:], in_=strayP[:])
                    strayM = spool.tile([1, 2 * ROWS], f32)
                    nc.vector.tensor_max(
                        out=strayM[:],
                        in0=strayC[0:1, 0:PPART:2],
                        in1=strayC[0:1, 1:PPART:2],
                    )
                    strayMM = spool.tile([1, 2 * ROWS], f32)
                    nc.vector.tensor_mul(
                        out=strayMM[:], in0=strayM[:], in1=maskS2_v
                    )
                    sv = strayMM[:].rearrange("one (r two) -> one r two", two=2)
                    nc.vector.reduce_sum(out=straysum[:], in_=sv, axis=X)

            # --- tail columns 12, 13 ----------------------------------------
            assert cursor == 0, cursor
            dC12 = dpool.tile([PPART, 1, HALF], f32, name="dC12", tag="dC12")
            q0(dC12[:], ptm[:, 12:13, :], 128, 0)
            dH1 = dpool.tile([PPART, 1, HALF // 2], f32, name="dH1", tag="dH1")
            q0(dH1[:], ptm[:, 13:14, 0 : HALF // 2], 128, 0)
            dH2 = dpool.tile([PPART, 1, HALF // 2], f32, name="dH2", tag="dH2")
            q0(dH2[0:112], ptm[0:112, 13:14, HALF // 2 : HALF], 112, 0)
            q0(dH2[112:128], ptm[112:128, 13:14, HALF // 2 : HALF], 16, 0)

            # Early tail for segments 0..5 (mid-stream, after red5).
            seg6 = spool.tile([PPART, NPAIR], f32)
            nc.vector.tensor_max(
                out=seg6[:],
                in0=stats12[:, 0 : 2 * NPAIR : 2],
                in1=stats12[:, 1 : 2 * NPAIR : 2],
            )
            masked6 = spool.tile([PPART, NPAIR], f32)
            nc.vector.tensor_mul(out=masked6[:], in0=seg6[:], in1=maskA_v[:, 0:NPAIR])
            partial6 = spool.tile([PPART, 1], f32)
            nc.vector.reduce_sum(out=partial6[:], in_=masked6[:], axis=X)

            sc12 = spool.tile([PPART, 1], f32)
            nc.vector.reduce_max(out=sc12[:], in_=dC12[:], axis=X)
            s13a = spool.tile([PPART, 1], f32)
            nc.vector.reduce_max(out=s13a[:], in_=dH1[:], axis=X)
            s13b = spool.tile([PPART, 1], f32)
            red_h2 = nc.vector.reduce_max(out=s13b[:], in_=dH2[:], axis=X)
            # dH2 has two writers; its own 16-desc final sub-load is the cover.
            cover_map.append((None, [red_h2]))  # resolved specially below

            # Segment 6 + combine.
            t13 = spool.tile([PPART, 1], f32)
            nc.vector.tensor_max(out=t13[:], in0=s13a[:], in1=s13b[:])
            seg7 = spool.tile([PPART, 1], f32)
            nc.vector.tensor_max(out=seg7[:], in0=sc12[:], in1=t13[:])
            masked7 = spool.tile([PPART, 1], f32)
            nc.vector.tensor_mul(
                out=masked7[:], in0=seg7[:], in1=maskA_v[:, NPAIR : NPAIR + 1]
            )
            partial = spool.tile([PPART, 1], f32)
            nc.vector.tensor_add(out=partial[:], in0=partial6[:], in1=masked7[:])

            acc = ppool.tile([1, ROWS], f32)
            nc.tensor.matmul(acc[:], partial[:], w1_v, start=True, stop=True)

            tmp = spool.tile([1, ROWS], f32)
            nc.vector.tensor_add(out=tmp[:], in0=acc[:], in1=straysum[:])
            res = spool.tile([1, ROWS], f32)
            nc.vector.tensor_mul(out=res[:], in0=tmp[:], in1=rcntT_v)
            nc.scalar.dma_start(out=out[:], in_=res[:])

    # dH2's reduce: keep only the wait belonging to the 16-desc final
    # sub-load (the higher accumulated value on whatever sems its two
    # writers got). If the two writers share a sem the merge handles it;
    # otherwise keep the later (higher-valued-by-program-order) one: the
    # final 16-desc load touches all lanes, so it covers the 112-desc one.
    special = cover_map.pop()  # (None, [red_h2])
    _rewrite_cover_waits(nc, cover_map, merge_rest=False)
    si = special[1][0].ins.sync_info
    if si is not None and len(si.on_wait) > 1:
        waits = sorted(si.on_wait, key=lambda w: w.wait_value)
        # waits attached in writer program order; the last writer is the
        # 16-desc cover. Identify it as the one whose sem matches the last
        # DMASW update before the reduce — simplest robust choice: keep the
        # wait added LAST (tile appends in writer order).
        si.on_wait = [list(si.on_wait)[-1]]
    # Final safety: no instruction may carry >1 wait.
    _rewrite_cover_waits(nc, [], merge_rest=True)
    return nc


def _get_nc():
    if "nc" not in _NC_CACHE:
        _NC_CACHE["nc"] = _build_nc()
    return _NC_CACHE["nc"]


def _host_layout():
    """Pair-aligned half-segment permutation and mask/weight constants.

    idx[p, j] = half-segment index (seg*2 + half, within one core's 1920)
    placed at (partition p, col j). Row r owns partitions 4r..4r+3; each
    holds 7 whole segments (cols 0..13, halves adjacent) plus one stray
    half at col 14 (segs 28/29 of the row, halves on partition pairs).
    """
    idx = np.empty((PPART, HPP), dtype=np.int64)
    w1row = np.zeros((PPART, ROWS), dtype=np.float32)
    for r in range(ROWS):
        for j in range(4):
            p = 4 * r + j
            w1row[p, r] = 1.0
            for k in range(NWHOLE):
                seg = r * NMEM + 7 * j + k
                idx[p, 2 * k] = 2 * seg
                idx[p, 2 * k + 1] = 2 * seg + 1
        idx[4 * r + 0, 14] = 2 * (r * NMEM + 28)
        idx[4 * r + 1, 14] = 2 * (r * NMEM + 28) + 1
        idx[4 * r + 2, 14] = 2 * (r * NMEM + 29)
        idx[4 * r + 3, 14] = 2 * (r * NMEM + 29) + 1
    ident = np.eye(PPART, dtype=np.float32)
    return idx.reshape(-1), w1row, ident


_IDX, _W1ROW, _IDENT = _host_layout()


def make_in_maps(ptm, mem_mask):
    ptm = np.ascontiguousarray(np.asarray(ptm, dtype=np.float32))
    mask = np.asarray(mem_mask).reshape(N, NMEM).astype(np.float32)
    halves = ptm.reshape(N * NMEM * 2, HALF)

    in_maps = []
    for i in range(NCORES):
        core_halves = halves[i * SEGS * 2 : (i + 1) * SEGS * 2]
        shard = core_halves[_IDX].reshape(PPART, HPP, HALF)
        m = mask[i * ROWS : (i + 1) * ROWS]  # (32, 30)
        maskA = np.empty((PPART, NWHOLE), dtype=np.float32)
        for j in range(4):
            maskA[j::4] = m[:, 7 * j : 7 * j + 7]
        consts = np.concatenate([_IDENT, _W1ROW, maskA], axis=1)
        consts = np.ascontiguousarray(consts, dtype=np.float32)
        # m1 = maskS2 | host-side 1/count on one partition.
        rcnt = (1.0 / m.sum(axis=1)).astype(np.float32)
        m1 = np.concatenate([m[:, 28:30].reshape(-1), rcnt]).reshape(1, -1)
        in_maps.append(
            {
                "ptm": shard,
                "consts": consts,
                "m1": np.ascontiguousarray(m1.astype(np.float32)),
            }
        )
    return in_maps


def _ensure_ntff_hook():
    """Register the axon NTFF profiling hook (the container's antenv lacks
    axon_hooks; synthesize it from trn_agent_boot), and stub the artifact
    upload which has no bucket access here."""
    import types

    try:
        from antenv.axon_hooks import get_axon_ntff_profile_hook  # noqa: F401
    except ImportError:
        import antenv
        from trn_agent_boot.trn_boot import _ntff_profile_via_ctypes

        mod = types.ModuleType("antenv.axon_hooks")
        mod._hook = _ntff_profile_via_ctypes("/opt/axon/libaxon_pjrt.so")
        mod.set_axon_ntff_profile_hook = lambda h: setattr(mod, "_hook", h)
        mod.get_axon_ntff_profile_hook = lambda: mod._hook
        sys.modules["antenv.axon_hooks"] = mod
        antenv.axon_hooks = mod

    from concourse import bass_utils

    if not getattr(bass_utils.upload_artifacts, "_stubbed", False):
        def _no_upload(tmpdir):
            return str(tmpdir)

        _no_upload._stubbed = True
        bass_utils.upload_artifacts = _no_upload


def run(ptm, mem_mask, trace=False):
    from concourse.bass_utils import run_bass_kernel_spmd

    if trace:
        _ensure_ntff_hook()

    in_maps = make_in_maps(ptm, mem_mask)

    nc = _get_nc()
    kr = run_bass_kernel_spmd(nc, in_maps, list(range(NCORES)), trace=trace)
    out = np.concatenate(
        [np.asarray(kr.results[i]["out"]).reshape(ROWS) for i in range(NCORES)]
    )
    return out.astype(np.float32), kr


def kernel(ptm, mem_mask):
    out, _ = run(ptm, mem_mask, trace=False)
    return out


# revision 7
# speedup vs baseline: 1.5277x; 1.5277x over previous
"""Trainium2 Bass kernel for BinOverlapPredictionFromMaxProj (segment max + masked mean).

Full computation:
  ptm: (32, 8, 30, 1, 72, 72) f32, mem_mask: (32, 8, 30) bool
  n = 32*8 = 256 rows; per row: max over 5184-feature axis per mem (30), then
  masked mean over mems -> out (256,) f32.

Sharding: data-parallel over the 256 fused rows across 8 cores (32 rows each).
Per core: 960 segments x 5184 features (~19.9 MB) -> memory-bound.

V3 design notes (on top of the V1 pair-aligned layout):

The 19.9 MB stream rides one SWDGE queue that fans descriptors over the
core's 16 DMA engines. Engine 15 (E79 on nc4) also serves the notification /
HW-dynamic queues and runs ~22% slower in bursts; with uniform round-robin it
builds an ~11us backlog that gates every load's completion semaphore (the
last data landed at ~68us instead of ~57us).

SWDGE lane-assignment rule (measured): for an n-descriptor DMA,
descs_per_lane = smallest divisor of n >= ceil(n/16); nlanes = n/d; lanes
are taken contiguously from a per-queue cursor that advances by nlanes mod
16 and persists across instructions (sem packets don't advance it).
Packet-speed rule (measured): n in {128, 126, 111, 15, 16} -> full 27 GB/s
per engine; n = 8 mod 16 (120, 104, 24) -> ~2x slower per packet; n = 112
(7 descs/lane) -> ~1.4x slower. So only "fast" shapes are used.

Exploits:
  - (126+2) skew pairs: the 126-desc load uses 14 lanes x 9 (full speed,
    skipping lanes cursor+14/15 = engines 14-15), the 2-desc remainder lands
    on those two lanes; advance 14+2 = 16 keeps the cursor parked at 0, so
    the starved lanes are always engines 14/15. 3 of 6 pairs are skewed
    (engine 15 gets ~46 instead of 60 packet-units; its ~22% deficit and
    burst interference fit in the slack), the rest stay uniform 128-desc
    so the fast engine 14 is not over-starved.
  - 16-desc 4-byte-descriptor re-reads of `consts` are all-lane "cover"
    loads: per-lane FIFO means a cover's completion implies every earlier
    descriptor on every lane has drained, so each skew pair's reduce can
    carry ONE sem wait (the cover's) instead of one per writer (walrus
    allows a single attached sync wait). A post-pass rewrites the
    tile-assigned multi-waits accordingly; a tiny DVE copy reads each cover
    tile so the cover gets a completion semaphore at all.

Tail: col 13 is loaded as two full-partition half-columns so the final
delivery-gated reduce is ~1.4us instead of ~2.8us, and the
pairmax/mask/rowsum for segments 0-5 runs mid-stream; only segment 6 and the
final matmul+mean remain after the last byte. 1/count is computed on host
and shipped in m1 (drops the DVE reciprocal and its table load).
"""

import sys

import numpy as np

if "/opt/trn_rl_repo" not in sys.path:
    sys.path.insert(0, "/opt/trn_rl_repo")

NCORES = 8
NF, NS, NMEM, FEAT = 32, 8, 30, 5184
N = NF * NS  # 256
ROWS = N // NCORES  # 32 rows per core
SEGS = ROWS * NMEM  # 960 segments per core
PPART = 128  # partitions
HALF = FEAT // 2  # 2592 floats per half-segment
HPP = SEGS * 2 // PPART  # 15 half-segments per partition
NWHOLE = 7  # whole segments per partition (cols 0..13)
NPAIR = 6  # column-pairs loaded with the skew structure (cols 0..11)
NC_ = PPART + ROWS + NWHOLE  # consts free dim: ident | w1 | maskA

_NC_CACHE = {}


def _nlanes(n):
    """Lanes used by an n-descriptor SWDGE DMA (measured ucode rule)."""
    need = -(-n // 16)
    for d in range(need, n + 1):
        if n % d == 0:
            return n // d
    return 1


def _patch_tile_drain():
    """Split the kernel-tail Drain's semaphore waits into standalone wait_ge
    instructions (one wait per instruction), to fit the walrus per-instruction
    sync-wait limit."""
    import concourse.tile as tile
    from concourse.vector_clock import ScopedClock

    if getattr(tile.TileContext._drain_and_barrier, "_single_wait_patch", False):
        return

    def _drain_and_barrier(self, tick_clock, wait_clock):
        drain_inst = self.nc.sync.drain()
        wait_clock.add_sem_waits(
            drain_inst.ins, ScopedClock({None: tick_clock.global_clock})
        )
        si = drain_inst.ins.sync_info
        waits = list(si.on_wait) if si is not None else []
        if len(waits) > 1:
            si.on_wait = [waits[0]]
            by_name = {h.name: h for h in self.sems.allocated().values()}
            for w in waits[1:]:
                self.nc.sync.wait_ge(by_name[w.ant_name], w.wait_value)

        self.nc.all_engine_barrier()
        assert self.sems is not None
        popped = self.nc._tile_sem_poison_stack.pop()
        assert popped is self._sem_poison
        self.nc.clear_and_free_semaphores(list(self.sems.allocated().values()))

    _drain_and_barrier._single_wait_patch = True
    tile.TileContext._drain_and_barrier = _drain_and_barrier


def _rewrite_cover_waits(nc, cover_map, merge_rest=True):
    """Post-pass: point each recorded reduce's sem waits at its cover load.

    cover_map: list of (cover_dma_bass_inst, [reduce_bass_inst, ...]).
    Per-lane FIFO on the SWDGE queue makes `cover delivered` imply `all
    earlier descriptors on every lane delivered`, so a single wait on the
    cover's accumulated sem value is a sound replacement for the
    tile-assigned one-wait-per-writer set (walrus allows only one).
    """
    # Accumulate DMA completion-sem values in program order.
    acc = {}
    cover_val = {}
    cover_ids = {id(c.ins): c for c, _ in cover_map}
    for fn in nc.m.functions:
        for b in fn.blocks:
            for ins in b.instructions:
                if type(ins).__name__ != "InstDMACopy":
                    continue
                si = ins.sync_info
                if si is None or not si.on_update:
                    continue
                for u in si.on_update:
                    if not u.ant_name.startswith("DMASW"):
                        continue
                    acc[u.id] = acc.get(u.id, 0) + u.update_value
                    if id(ins) in cover_ids:
                        cover_val[id(ins)] = (u.ant_name, u.id, acc[u.id])
    for cover, reds in cover_map:
        key = id(cover.ins)
        assert key in cover_val, f"cover {cover.ins.name} got no DMASW sem"
        ant_name, sem_id, val = cover_val[key]
        for r in reds:
            si = r.ins.sync_info
            assert si is not None and si.on_wait, f"{r.ins.name} has no waits"
            w = si.on_wait[0]
            w.ant_name = ant_name
            w.id = sem_id
            w.wait_value = val
            si.on_wait = [w]
    if merge_rest:
        for fn in nc.m.functions:
            for b in fn.blocks:
                for ins in b.instructions:
                    si = ins.sync_info
                    if si is None or len(si.on_wait) <= 1:
                        continue
                    waits = list(si.on_wait)
                    names = {w.ant_name for w in waits}
                    assert len(names) == 1, (
                        f"unresolved multi-sem wait on {ins.name}: "
                        f"{[(w.ant_name, w.wait_value) for w in waits]}"
                    )
                    best = max(waits, key=lambda w: w.wait_value)
                    si.on_wait = [best]


def _build_nc():
    import concourse.bass as bass
    import concourse.tile as tile
    from concourse import mybir
    from concourse.bass import MemorySpace

    _patch_tile_drain()

    f32 = mybir.dt.float32
    X = mybir.AxisListType.X

    nc = bass.Bass("TRN2")
    ptm = nc.dram_tensor("ptm", [PPART, HPP, HALF], f32, kind="ExternalInput")
    consts = nc.dram_tensor("consts", [PPART, NC_], f32, kind="ExternalInput")
    m1 = nc.dram_tensor("m1", [1, 2 * ROWS + ROWS], f32, kind="ExternalInput")
    out = nc.dram_tensor("out", [1, ROWS], f32, kind="ExternalOutput")

    cover_map = []  # (cover dma inst, [reduces to rewrite])
    cursor = 0  # SWDGE lane cursor (relative; engine 15 = cursor 15)

    def q0(dst, src, ndesc, expect_adv):
        nonlocal cursor
        inst = nc.gpsimd.dma_start(out=dst, in_=src)
        adv = _nlanes(ndesc) % 16
        assert adv == expect_adv % 16, (ndesc, adv, expect_adv)
        cursor = (cursor + adv) % 16
        return inst

    with tile.TileContext(nc) as tc:
        with (
            tc.tile_pool(name="data", bufs=1) as dpool,
            tc.tile_pool(name="small", bufs=1) as spool,
            tc.tile_pool(name="psum", bufs=1, space=MemorySpace.PSUM) as ppool,
        ):
            # --- early loads -------------------------------------------------
            const_t = spool.tile([PPART, NC_], f32)
            q0(const_t[:], consts[:], 128, 0)
            ident_v = const_t[:, 0:PPART]
            w1_v = const_t[:, PPART : PPART + ROWS]
            maskA_v = const_t[:, PPART + ROWS : NC_]

            m1_t = spool.tile([1, 2 * ROWS + ROWS], f32)
            nc.scalar.dma_start(out=m1_t[:], in_=m1[:])
            maskS2_v = m1_t[0:1, 0 : 2 * ROWS]
            rcntT_v = m1_t[0:1, 2 * ROWS : 2 * ROWS + ROWS]

            # junk tiles for repair/cover reads (4B descriptors)
            jr = spool.tile([16, 16], f32)
            jread = spool.tile([1, 8], f32)

            # PE warmup: touch const tile so later PE ops carry one data wait.
            warm = ppool.tile([1, ROWS], f32)
            nc.tensor.matmul(warm[:], const_t[:, 0:1], const_t[:, 0:ROWS],
                             start=True, stop=True)

            # Stray (half-segment) column, full uniform load.
            dS = dpool.tile([PPART, 1, HALF], f32, name="dataS", tag="dataS")
            q0(dS[:], ptm[:, HPP - 1 : HPP, :], 128, 0)
            statS = spool.tile([PPART, 1], f32)
            nc.vector.reduce_max(out=statS[:], in_=dS[:], axis=X)

            # DVE warm-touch + power-ramp burn in the pre-stream window.
            touch = spool.tile([1, 1], f32)
            nc.vector.tensor_copy(out=touch[:], in_=const_t[0:1, 0:1])
            touch2 = spool.tile([1, 1], f32)
            nc.vector.tensor_copy(out=touch2[:], in_=m1_t[0:1, 0:1])
            burn = spool.tile([PPART, 1], f32)
            nc.vector.reduce_max(out=burn[:], in_=const_t[:], axis=X)
            nc.vector.reduce_max(out=burn[:], in_=const_t[:], axis=X)

            # PE transpose of the stray half-maxes to one partition.
            strayP = ppool.tile([1, PPART], f32)
            nc.tensor.transpose(strayP[:], statS[:], ident_v)

            # --- main pair stream: skew (126+2) / uniform alternating -------
            stats12 = spool.tile([PPART, 2 * NPAIR], f32)
            straysum = spool.tile([1, ROWS], f32)
            for k in range(NPAIR):
                d = dpool.tile([PPART, 2, HALF], f32, name=f"dP{k}", tag=f"dP{k}")
                src = ptm[:, 2 * k : 2 * k + 2, :]
                assert cursor == 0, cursor
                if k % 2 == 0:
                    # skew pair: 126-desc main (14 lanes x 9, skipping
                    # engines 14/15) + 2-desc remainder on engines 14/15.
                    q0(d[0:126], src[0:126], 126, 14)
                    q0(d[126:128], src[126:128], 2, 2)
                    cov = q0(jr[0:16, 8 + k : 9 + k], consts[0:16, 0:1], 16, 0)
                    # Reader forces a completion sem onto the cover load.
                    cread = nc.vector.tensor_copy(
                        out=jread[0:1, k : k + 1], in_=jr[0:1, 8 + k : 9 + k]
                    )
                    red = nc.vector.reduce_max(
                        out=stats12[:, 2 * k : 2 * k + 2], in_=d[:], axis=X
                    )
                    cover_map.append((cov, [cread, red]))
                else:
                    q0(d[:], src[:], 128, 0)
                    nc.vector.reduce_max(
                        out=stats12[:, 2 * k : 2 * k + 2], in_=d[:], axis=X
                    )

                if k == 1:
                    # Stray path on DVE, mid-stream (off critical path).
                    strayC = spool.tile([1, PPART], f32)
                    nc.vector.tensor_copy(out=strayC[:], in_=strayP[:])
                    strayM = spool.tile([1, 2 * ROWS], f32)
                    nc.vector.tensor_max(
                        out=strayM[:],
                        in0=strayC[0:1, 0:PPART:2],
                        in1=strayC[0:1, 1:PPART:2],
                    )
                    strayMM = spool.tile([1, 2 * ROWS], f32)
                    nc.vector.tensor_mul(
                        out=strayMM[:], in0=strayM[:], in1=maskS2_v
                    )
                    sv = strayMM[:].rearrange("one (r two) -> one r two", two=2)
                    nc.vector.reduce_sum(out=straysum[:], in_=sv, axis=X)

            # --- tail columns 12, 13 ----------------------------------------
            assert cursor == 0, cursor
            dC12 = dpool.tile([PPART, 1, HALF], f32, name="dC12", tag="dC12")
            q0(dC12[:], ptm[:, 12:13, :], 128, 0)
            dH1 = dpool.tile([PPART, 1, HALF // 2], f32, name="dH1", tag="dH1")
            q0(dH1[:], ptm[:, 13:14, 0 : HALF // 2], 128, 0)
            dH2 = dpool.tile([PPART, 1, HALF // 2], f32, name="dH2", tag="dH2")
            q0(dH2[:], ptm[:, 13:14, HALF // 2 : HALF], 128, 0)

            # Early tail for segments 0..5 (mid-stream, after red5).
            seg6 = spool.tile([PPART, NPAIR], f32)
            nc.vector.tensor_max(
                out=seg6[:],
                in0=stats12[:, 0 : 2 * NPAIR : 2],
                in1=stats12[:, 1 : 2 * NPAIR : 2],
            )
            masked6 = spool.tile([PPART, NPAIR], f32)
            nc.vector.tensor_mul(out=masked6[:], in0=seg6[:], in1=maskA_v[:, 0:NPAIR])
            partial6 = spool.tile([PPART, 1], f32)
            nc.vector.reduce_sum(out=partial6[:], in_=masked6[:], axis=X)

            sc12 = spool.tile([PPART, 1], f32)
            nc.vector.reduce_max(out=sc12[:], in_=dC12[:], axis=X)
            s13a = spool.tile([PPART, 1], f32)
            nc.vector.reduce_max(out=s13a[:], in_=dH1[:], axis=X)
            s13b = spool.tile([PPART, 1], f32)
            nc.vector.reduce_max(out=s13b[:], in_=dH2[:], axis=X)

            # Segment 6 + combine.
            t13 = spool.tile([PPART, 1], f32)
            nc.vector.tensor_max(out=t13[:], in0=s13a[:], in1=s13b[:])
            seg7 = spool.tile([PPART, 1], f32)
            nc.vector.tensor_max(out=seg7[:], in0=sc12[:], in1=t13[:])
            masked7 = spool.tile([PPART, 1], f32)
            nc.vector.tensor_mul(
                out=masked7[:], in0=seg7[:], in1=maskA_v[:, NPAIR : NPAIR + 1]
            )
            partial = spool.tile([PPART, 1], f32)
            nc.vector.tensor_add(out=partial[:], in0=partial6[:], in1=masked7[:])

            acc = ppool.tile([1, ROWS], f32)
            nc.tensor.matmul(acc[:], partial[:], w1_v, start=True, stop=True)

            tmp = spool.tile([1, ROWS], f32)
            nc.vector.tensor_add(out=tmp[:], in0=acc[:], in1=straysum[:])
            res = spool.tile([1, ROWS], f32)
            nc.vector.tensor_mul(out=res[:], in0=tmp[:], in1=rcntT_v)
            nc.scalar.dma_start(out=out[:], in_=res[:])

    _rewrite_cover_waits(nc, cover_map, merge_rest=True)
    return nc


def _get_nc():
    if "nc" not in _NC_CACHE:
        _NC_CACHE["nc"] = _build_nc()
    return _NC_CACHE["nc"]


def _host_layout():
    """Pair-aligned half-segment permutation and mask/weight constants.

    idx[p, j] = half-segment index (seg*2 + half, within one core's 1920)
    placed at (partition p, col j). Row r owns partitions 4r..4r+3; each
    holds 7 whole segments (cols 0..13, halves adjacent) plus one stray
    half at col 14 (segs 28/29 of the row, halves on partition pairs).
    """
    idx = np.empty((PPART, HPP), dtype=np.int64)
    w1row = np.zeros((PPART, ROWS), dtype=np.float32)
    for r in range(ROWS):
        for j in range(4):
            p = 4 * r + j
            w1row[p, r] = 1.0
            for k in range(NWHOLE):
                seg = r * NMEM + 7 * j + k
                idx[p, 2 * k] = 2 * seg
                idx[p, 2 * k + 1] = 2 * seg + 1
        idx[4 * r + 0, 14] = 2 * (r * NMEM + 28)
        idx[4 * r + 1, 14] = 2 * (r * NMEM + 28) + 1
        idx[4 * r + 2, 14] = 2 * (r * NMEM + 29)
        idx[4 * r + 3, 14] = 2 * (r * NMEM + 29) + 1
    ident = np.eye(PPART, dtype=np.float32)
    return idx.reshape(-1), w1row, ident


_IDX, _W1ROW, _IDENT = _host_layout()


def make_in_maps(ptm, mem_mask):
    ptm = np.ascontiguousarray(np.asarray(ptm, dtype=np.float32))
    mask = np.asarray(mem_mask).reshape(N, NMEM).astype(np.float32)
    halves = ptm.reshape(N * NMEM * 2, HALF)

    in_maps = []
    for i in range(NCORES):
        core_halves = halves[i * SEGS * 2 : (i + 1) * SEGS * 2]
        shard = core_halves[_IDX].reshape(PPART, HPP, HALF)
        m = mask[i * ROWS : (i + 1) * ROWS]  # (32, 30)
        maskA = np.empty((PPART, NWHOLE), dtype=np.float32)
        for j in range(4):
            maskA[j::4] = m[:, 7 * j : 7 * j + 7]
        consts = np.concatenate([_IDENT, _W1ROW, maskA], axis=1)
        consts = np.ascontiguousarray(consts, dtype=np.float32)
        # m1 = maskS2 | host-side 1/count on one partition.
        rcnt = (1.0 / m.sum(axis=1)).astype(np.float32)
        m1 = np.concatenate([m[:, 28:30].reshape(-1), rcnt]).reshape(1, -1)
        in_maps.append(
            {
                "ptm": shard,
                "consts": consts,
                "m1": np.ascontiguousarray(m1.astype(np.float32)),
            }
        )
    return in_maps


def _ensure_ntff_hook():
    """Register the axon NTFF profiling hook (the container's antenv lacks
    axon_hooks; synthesize it from trn_agent_boot), and stub the artifact
    upload which has no bucket access here."""
    import types

    try:
        from antenv.axon_hooks import get_axon_ntff_profile_hook  # noqa: F401
    except ImportError:
        import antenv
        from trn_agent_boot.trn_boot import _ntff_profile_via_ctypes

        mod = types.ModuleType("antenv.axon_hooks")
        mod._hook = _ntff_profile_via_ctypes("/opt/axon/libaxon_pjrt.so")
        mod.set_axon_ntff_profile_hook = lambda h: setattr(mod, "_hook", h)
        mod.get_axon_ntff_profile_hook = lambda: mod._hook
        sys.modules["antenv.axon_hooks"] = mod
        antenv.axon_hooks = mod

    from concourse import bass_utils

    if not getattr(bass_utils.upload_artifacts, "_stubbed", False):
        def _no_upload(tmpdir):
            return str(tmpdir)

        _no_upload._stubbed = True
        bass_utils.upload_artifacts = _no_upload


def run(ptm, mem_mask, trace=False):
    from concourse.bass_utils import run_bass_kernel_spmd

    if trace:
        _ensure_ntff_hook()

    in_maps = make_in_maps(ptm, mem_mask)

    nc = _get_nc()
    kr = run_bass_kernel_spmd(nc, in_maps, list(range(NCORES)), trace=trace)
    out = np.concatenate(
        [np.asarray(kr.results[i]["out"]).reshape(ROWS) for i in range(NCORES)]
    )
    return out.astype(np.float32), kr


def kernel(ptm, mem_mask):
    out, _ = run(ptm, mem_mask, trace=False)
    return out


# revision 9
# speedup vs baseline: 1.7841x; 1.1678x over previous
"""Trainium2 Bass kernel for BinOverlapPredictionFromMaxProj (segment max + masked mean).

Full computation:
  ptm: (32, 8, 30, 1, 72, 72) f32, mem_mask: (32, 8, 30) bool
  n = 32*8 = 256 rows; per row: max over 5184-feature axis per mem (30), then
  masked mean over mems -> out (256,) f32.

Sharding: data-parallel over the 256 fused rows across 8 cores (32 rows each).
Per core: 960 segments x 5184 features (~19.9 MB) -> memory-bound.

V3 design notes (on top of the V1 pair-aligned layout):

The 19.9 MB stream rides one SWDGE queue that fans descriptors over the
core's 16 DMA engines. Engine 15 (E79 on nc4) also serves the notification /
HW-dynamic queues and runs ~22% slower in bursts; with uniform round-robin it
builds an ~11us backlog that gates every load's completion semaphore (the
last data landed at ~68us instead of ~57us).

SWDGE lane-assignment rule (measured): for an n-descriptor DMA,
descs_per_lane = smallest divisor of n >= ceil(n/16); nlanes = n/d; lanes
are taken contiguously from a per-queue cursor that advances by nlanes mod
16 and persists across instructions (sem packets don't advance it).
Packet-speed rule (measured): n in {128, 126, 111, 15, 16} -> full 27 GB/s
per engine; n = 8 mod 16 (120, 104, 24) -> ~2x slower per packet; n = 112
(7 descs/lane) -> ~1.4x slower. So only "fast" shapes are used.

Exploits:
  - (126+2) skew pairs: the 126-desc load uses 14 lanes x 9 (full speed,
    skipping lanes cursor+14/15 = engines 14-15), the 2-desc remainder lands
    on those two lanes; advance 14+2 = 16 keeps the cursor parked at 0, so
    the starved lanes are always engines 14/15. 3 of 6 pairs are skewed
    (engine 15 gets ~46 instead of 60 packet-units; its ~22% deficit and
    burst interference fit in the slack), the rest stay uniform 128-desc
    so the fast engine 14 is not over-starved.
  - 16-desc 4-byte-descriptor re-reads of `consts` are all-lane "cover"
    loads: per-lane FIFO means a cover's completion implies every earlier
    descriptor on every lane has drained, so each skew pair's reduce can
    carry ONE sem wait (the cover's) instead of one per writer (walrus
    allows a single attached sync wait). A post-pass rewrites the
    tile-assigned multi-waits accordingly; a tiny DVE copy reads each cover
    tile so the cover gets a completion semaphore at all.

Tail: col 13 is loaded as two full-partition half-columns so the final
delivery-gated reduce is ~1.4us instead of ~2.8us, and the
pairmax/mask/rowsum for segments 0-5 runs mid-stream; only segment 6 and the
final matmul+mean remain after the last byte. 1/count is computed on host
and shipped in m1 (drops the DVE reciprocal and its table load).
"""

import sys

import numpy as np

if "/opt/trn_rl_repo" not in sys.path:
    sys.path.insert(0, "/opt/trn_rl_repo")

NCORES = 8
NF, NS, NMEM, FEAT = 32, 8, 30, 5184
N = NF * NS  # 256
ROWS = N // NCORES  # 32 rows per core
SEGS = ROWS * NMEM  # 960 segments per core
PPART = 128  # partitions
HALF = FEAT // 2  # 2592 floats per half-segment
HPP = SEGS * 2 // PPART  # 15 half-segments per partition
NWHOLE = 7  # whole segments per partition (cols 0..13)
NPAIR = 6  # column-pairs loaded with the skew structure (cols 0..11)
NC_ = PPART + ROWS + NWHOLE  # consts free dim: ident | w1 | maskA

_NC_CACHE = {}


def _nlanes(n):
    """Lanes used by an n-descriptor SWDGE DMA (measured ucode rule)."""
    need = -(-n // 16)
    for d in range(need, n + 1):
        if n % d == 0:
            return n // d
    return 1


def _patch_tile_drain():
    """Split the kernel-tail Drain's semaphore waits into standalone wait_ge
    instructions (one wait per instruction), to fit the walrus per-instruction
    sync-wait limit."""
    import concourse.tile as tile
    from concourse.vector_clock import ScopedClock

    if getattr(tile.TileContext._drain_and_barrier, "_single_wait_patch", False):
        return

    def _drain_and_barrier(self, tick_clock, wait_clock):
        drain_inst = self.nc.sync.drain()
        wait_clock.add_sem_waits(
            drain_inst.ins, ScopedClock({None: tick_clock.global_clock})
        )
        si = drain_inst.ins.sync_info
        waits = list(si.on_wait) if si is not None else []
        if len(waits) > 1:
            si.on_wait = [waits[0]]
            by_name = {h.name: h for h in self.sems.allocated().values()}
            for w in waits[1:]:
                self.nc.sync.wait_ge(by_name[w.ant_name], w.wait_value)

        self.nc.all_engine_barrier()
        assert self.sems is not None
        popped = self.nc._tile_sem_poison_stack.pop()
        assert popped is self._sem_poison
        self.nc.clear_and_free_semaphores(list(self.sems.allocated().values()))

    _drain_and_barrier._single_wait_patch = True
    tile.TileContext._drain_and_barrier = _drain_and_barrier


def _rewrite_cover_waits(nc, cover_map, merge_rest=True):
    """Post-pass: point each recorded reduce's sem waits at its cover load.

    cover_map: list of (cover_dma_bass_inst, [reduce_bass_inst, ...]).
    Per-lane FIFO on the SWDGE queue makes `cover delivered` imply `all
    earlier descriptors on every lane delivered`, so a single wait on the
    cover's accumulated sem value is a sound replacement for the
    tile-assigned one-wait-per-writer set (walrus allows only one).
    """
    # Accumulate DMA completion-sem values in program order.
    acc = {}
    cover_val = {}
    cover_ids = {id(c.ins): c for c, _ in cover_map}
    for fn in nc.m.functions:
        for b in fn.blocks:
            for ins in b.instructions:
                if type(ins).__name__ != "InstDMACopy":
                    continue
                si = ins.sync_info
                if si is None or not si.on_update:
                    continue
                for u in si.on_update:
                    if not u.ant_name.startswith("DMASW"):
                        continue
                    acc[u.id] = acc.get(u.id, 0) + u.update_value
                    if id(ins) in cover_ids:
                        cover_val[id(ins)] = (u.ant_name, u.id, acc[u.id])
    for cover, reds in cover_map:
        key = id(cover.ins)
        assert key in cover_val, f"cover {cover.ins.name} got no DMASW sem"
        ant_name, sem_id, val = cover_val[key]
        for r in reds:
            si = r.ins.sync_info
            assert si is not None and si.on_wait, f"{r.ins.name} has no waits"
            w = si.on_wait[0]
            w.ant_name = ant_name
            w.id = sem_id
            w.wait_value = val
            si.on_wait = [w]
    if merge_rest:
        for fn in nc.m.functions:
            for b in fn.blocks:
                for ins in b.instructions:
                    si = ins.sync_info
                    if si is None or len(si.on_wait) <= 1:
                        continue
                    waits = list(si.on_wait)
                    names = {w.ant_name for w in waits}
                    assert len(names) == 1, (
                        f"unresolved multi-sem wait on {ins.name}: "
                        f"{[(w.ant_name, w.wait_value) for w in waits]}"
                    )
                    best = max(waits, key=lambda w: w.wait_value)
                    si.on_wait = [best]


def _build_nc():
    import concourse.bass as bass
    import concourse.tile as tile
    from concourse import mybir
    from concourse.bass import MemorySpace

    _patch_tile_drain()

    f32 = mybir.dt.float32
    X = mybir.AxisListType.X

    nc = bass.Bass("TRN2")
    ptm = nc.dram_tensor("ptm", [PPART, HPP, HALF], f32, kind="ExternalInput")
    consts = nc.dram_tensor("consts", [PPART, NC_], f32, kind="ExternalInput")
    m1 = nc.dram_tensor("m1", [1, 2 * ROWS + ROWS], f32, kind="ExternalInput")
    out = nc.dram_tensor("out", [1, ROWS], f32, kind="ExternalOutput")

    cover_map = []  # (cover dma inst, [reduces to rewrite])
    cursor = 0  # SWDGE lane cursor (relative; engine 15 = cursor 15)

    def q0(dst, src, ndesc, expect_adv):
        nonlocal cursor
        inst = nc.gpsimd.dma_start(out=dst, in_=src)
        adv = _nlanes(ndesc) % 16
        assert adv == expect_adv % 16, (ndesc, adv, expect_adv)
        cursor = (cursor + adv) % 16
        return inst

    with tile.TileContext(nc) as tc:
        with (
            tc.tile_pool(name="data", bufs=1) as dpool,
            tc.tile_pool(name="small", bufs=1) as spool,
            tc.tile_pool(name="psum", bufs=1, space=MemorySpace.PSUM) as ppool,
        ):
            # --- early loads -------------------------------------------------
            const_t = spool.tile([PPART, NC_], f32)
            q0(const_t[:], consts[:], 128, 0)
            ident_v = const_t[:, 0:PPART]
            w1_v = const_t[:, PPART : PPART + ROWS]
            maskA_v = const_t[:, PPART + ROWS : NC_]

            m1_t = spool.tile([1, 2 * ROWS + ROWS], f32)
            nc.scalar.dma_start(out=m1_t[:], in_=m1[:])
            maskS2_v = m1_t[0:1, 0 : 2 * ROWS]
            rcntT_v = m1_t[0:1, 2 * ROWS : 2 * ROWS + ROWS]

            # PE warmup: touch const tile so later PE ops carry one data wait.
            warm = ppool.tile([1, ROWS], f32)
            nc.tensor.matmul(warm[:], const_t[:, 0:1], const_t[:, 0:ROWS],
                             start=True, stop=True)

            # Stray (half-segment) column, full uniform load.
            dS = dpool.tile([PPART, 1, HALF], f32, name="dataS", tag="dataS")
            q0(dS[:], ptm[:, HPP - 1 : HPP, :], 128, 0)
            statS = spool.tile([PPART, 1], f32)
            nc.vector.reduce_max(out=statS[:], in_=dS[:], axis=X)

            # DVE warm-touch + power-ramp burn in the pre-stream window.
            touch = spool.tile([1, 1], f32)
            nc.vector.tensor_copy(out=touch[:], in_=const_t[0:1, 0:1])
            touch2 = spool.tile([1, 1], f32)
            nc.vector.tensor_copy(out=touch2[:], in_=m1_t[0:1, 0:1])
            burn = spool.tile([PPART, 1], f32)
            nc.vector.reduce_max(out=burn[:], in_=const_t[:], axis=X)
            nc.vector.reduce_max(out=burn[:], in_=const_t[:], axis=X)

            # PE transpose of the stray half-maxes to one partition.
            strayP = ppool.tile([1, PPART], f32)
            nc.tensor.transpose(strayP[:], statS[:], ident_v)

            # --- main pair stream: uniform 128-desc loads (the only shape
            # measured to sustain full per-engine speed under streaming
            # concurrency; see probe notes in the docstring) ---------------
            stats12 = spool.tile([PPART, 2 * NPAIR], f32)
            straysum = spool.tile([1, ROWS], f32)
            for k in range(NPAIR):
                d = dpool.tile([PPART, 2, HALF], f32, name=f"dP{k}", tag=f"dP{k}")
                src = ptm[:, 2 * k : 2 * k + 2, :]
                assert cursor == 0, cursor
                q0(d[:], src[:], 128, 0)
                nc.vector.reduce_max(
                    out=stats12[:, 2 * k : 2 * k + 2], in_=d[:], axis=X
                )

                if k == 1:
                    # Stray path on DVE, mid-stream (off critical path).
                    strayC = spool.tile([1, PPART], f32)
                    nc.vector.tensor_copy(out=strayC[:], in_=strayP[:])
                    strayM = spool.tile([1, 2 * ROWS], f32)
                    nc.vector.tensor_max(
                        out=strayM[:],
                        in0=strayC[0:1, 0:PPART:2],
                        in1=strayC[0:1, 1:PPART:2],
                    )
                    strayMM = spool.tile([1, 2 * ROWS], f32)
                    nc.vector.tensor_mul(
                        out=strayMM[:], in0=strayM[:], in1=maskS2_v
                    )
                    sv = strayMM[:].rearrange("one (r two) -> one r two", two=2)
                    nc.vector.reduce_sum(out=straysum[:], in_=sv, axis=X)

            # --- tail columns 12, 13 ----------------------------------------
            assert cursor == 0, cursor
            dC12 = dpool.tile([PPART, 1, HALF], f32, name="dC12", tag="dC12")
            q0(dC12[:], ptm[:, 12:13, :], 128, 0)
            dH1 = dpool.tile([PPART, 1, HALF // 2], f32, name="dH1", tag="dH1")
            q0(dH1[:], ptm[:, 13:14, 0 : HALF // 2], 128, 0)
            dH2 = dpool.tile([PPART, 1, HALF // 2], f32, name="dH2", tag="dH2")
            q0(dH2[:], ptm[:, 13:14, HALF // 2 : HALF], 128, 0)

            # Early tail for segments 0..5 (mid-stream, after red5).
            seg6 = spool.tile([PPART, NPAIR], f32)
            nc.vector.tensor_max(
                out=seg6[:],
                in0=stats12[:, 0 : 2 * NPAIR : 2],
                in1=stats12[:, 1 : 2 * NPAIR : 2],
            )
            masked6 = spool.tile([PPART, NPAIR], f32)
            nc.vector.tensor_mul(out=masked6[:], in0=seg6[:], in1=maskA_v[:, 0:NPAIR])
            partial6 = spool.tile([PPART, 1], f32)
            nc.vector.reduce_sum(out=partial6[:], in_=masked6[:], axis=X)

            sc12 = spool.tile([PPART, 1], f32)
            nc.vector.reduce_max(out=sc12[:], in_=dC12[:], axis=X)
            s13a = spool.tile([PPART, 1], f32)
            nc.vector.reduce_max(out=s13a[:], in_=dH1[:], axis=X)
            s13b = spool.tile([PPART, 1], f32)
            nc.vector.reduce_max(out=s13b[:], in_=dH2[:], axis=X)

            # Segment 6 + combine.
            t13 = spool.tile([PPART, 1], f32)
            nc.vector.tensor_max(out=t13[:], in0=s13a[:], in1=s13b[:])
            seg7 = spool.tile([PPART, 1], f32)
            nc.vector.tensor_max(out=seg7[:], in0=sc12[:], in1=t13[:])
            masked7 = spool.tile([PPART, 1], f32)
            nc.vector.tensor_mul(
                out=masked7[:], in0=seg7[:], in1=maskA_v[:, NPAIR : NPAIR + 1]
            )
            partial = spool.tile([PPART, 1], f32)
            nc.vector.tensor_add(out=partial[:], in0=partial6[:], in1=masked7[:])

            acc = ppool.tile([1, ROWS], f32)
            nc.tensor.matmul(acc[:], partial[:], w1_v, start=True, stop=True)

            tmp = spool.tile([1, ROWS], f32)
            nc.vector.tensor_add(out=tmp[:], in0=acc[:], in1=straysum[:])
            res = spool.tile([1, ROWS], f32)
            nc.vector.tensor_mul(out=res[:], in0=tmp[:], in1=rcntT_v)
            nc.scalar.dma_start(out=out[:], in_=res[:])

    _rewrite_cover_waits(nc, cover_map, merge_rest=True)
    return nc


def _get_nc():
    if "nc" not in _NC_CACHE:
        _NC_CACHE["nc"] = _build_nc()
    return _NC_CACHE["nc"]


def _host_layout():
    """Pair-aligned half-segment permutation and mask/weight constants.

    idx[p, j] = half-segment index (seg*2 + half, within one core's 1920)
    placed at (partition p, col j). Row r owns partitions 4r..4r+3; each
    holds 7 whole segments (cols 0..13, halves adjacent) plus one stray
    half at col 14 (segs 28/29 of the row, halves on partition pairs).
    """
    idx = np.empty((PPART, HPP), dtype=np.int64)
    w1row = np.zeros((PPART, ROWS), dtype=np.float32)
    for r in range(ROWS):
        for j in range(4):
            p = 4 * r + j
            w1row[p, r] = 1.0
            for k in range(NWHOLE):
                seg = r * NMEM + 7 * j + k
                idx[p, 2 * k] = 2 * seg
                idx[p, 2 * k + 1] = 2 * seg + 1
        idx[4 * r + 0, 14] = 2 * (r * NMEM + 28)
        idx[4 * r + 1, 14] = 2 * (r * NMEM + 28) + 1
        idx[4 * r + 2, 14] = 2 * (r * NMEM + 29)
        idx[4 * r + 3, 14] = 2 * (r * NMEM + 29) + 1
    ident = np.eye(PPART, dtype=np.float32)
    return idx.reshape(-1), w1row, ident


_IDX, _W1ROW, _IDENT = _host_layout()


def make_in_maps(ptm, mem_mask):
    ptm = np.ascontiguousarray(np.asarray(ptm, dtype=np.float32))
    mask = np.asarray(mem_mask).reshape(N, NMEM).astype(np.float32)
    halves = ptm.reshape(N * NMEM * 2, HALF)

    in_maps = []
    for i in range(NCORES):
        core_halves = halves[i * SEGS * 2 : (i + 1) * SEGS * 2]
        shard = core_halves[_IDX].reshape(PPART, HPP, HALF)
        m = mask[i * ROWS : (i + 1) * ROWS]  # (32, 30)
        maskA = np.empty((PPART, NWHOLE), dtype=np.float32)
        for j in range(4):
            maskA[j::4] = m[:, 7 * j : 7 * j + 7]
        consts = np.concatenate([_IDENT, _W1ROW, maskA], axis=1)
        consts = np.ascontiguousarray(consts, dtype=np.float32)
        # m1 = maskS2 | host-side 1/count on one partition.
        rcnt = (1.0 / m.sum(axis=1)).astype(np.float32)
        m1 = np.concatenate([m[:, 28:30].reshape(-1), rcnt]).reshape(1, -1)
        in_maps.append(
            {
                "ptm": shard,
                "consts": consts,
                "m1": np.ascontiguousarray(m1.astype(np.float32)),
            }
        )
    return in_maps


def _ensure_ntff_hook():
    """Register the axon NTFF profiling hook (the container's antenv lacks
    axon_hooks; synthesize it from trn_agent_boot), and stub the artifact
    upload which has no bucket access here."""
    import types

    try:
        from antenv.axon_hooks import get_axon_ntff_profile_hook  # noqa: F401
    except ImportError:
        import antenv
        from trn_agent_boot.trn_boot import _ntff_profile_via_ctypes

        mod = types.ModuleType("antenv.axon_hooks")
        mod._hook = _ntff_profile_via_ctypes("/opt/axon/libaxon_pjrt.so")
        mod.set_axon_ntff_profile_hook = lambda h: setattr(mod, "_hook", h)
        mod.get_axon_ntff_profile_hook = lambda: mod._hook
        sys.modules["antenv.axon_hooks"] = mod
        antenv.axon_hooks = mod

    from concourse import bass_utils

    if not getattr(bass_utils.upload_artifacts, "_stubbed", False):
        def _no_upload(tmpdir):
            return str(tmpdir)

        _no_upload._stubbed = True
        bass_utils.upload_artifacts = _no_upload


def run(ptm, mem_mask, trace=False):
    from concourse.bass_utils import run_bass_kernel_spmd

    if trace:
        _ensure_ntff_hook()

    in_maps = make_in_maps(ptm, mem_mask)

    nc = _get_nc()
    kr = run_bass_kernel_spmd(nc, in_maps, list(range(NCORES)), trace=trace)
    out = np.concatenate(
        [np.asarray(kr.results[i]["out"]).reshape(ROWS) for i in range(NCORES)]
    )
    return out.astype(np.float32), kr


def kernel(ptm, mem_mask):
    out, _ = run(ptm, mem_mask, trace=False)
    return out
